# revision 21
# baseline (speedup 1.0000x reference)
"""Trainium2 Bass kernel for a 2-layer GCN + global mean pool + FC.

Strategy (8 NeuronCores, SPMD single NEFF):
  - Nodes (and their in-edges) partitioned by dst across 8 cores; weights
    replicated; h1 shards AllGathered between layers; pooled sums AllReduced.
  - Gather sources are PRESCALED by dinv_src on the host (x' = d^-1/2 x in
    bf16, 256B rows), so gathered tiles feed the TensorEngine scatter matmuls
    directly -- no per-edge DVE scale/convert pass at all.  The remaining
    per-dst factor d^-1/2 is applied in the per-block epilogue as a
    per-partition scalar on the Scalar engine; the bias rides an augmented
    ones-row in the weight matmul.
  - Per 128-edge chunk, one-hot masks S (0/1, bf16, generated by a batched
    is_equal on DVE) scatter-add the gathered rows into PSUM via TensorE:
    agg[128d,64f] += S[e,d].T @ g[e,0:64].
  - Layer-2 self-loop rows are kept SBUF-resident (h1'' = dinv^2*h1 written
    during the layer-1 epilogue) -- no DMA round trip.
  - Chunk slots inside each supergather are laid out round-robin by block
    (per-core padding chunks cluster at the tail of each gather; padding
    entries gather row 0 and are masked out by all-zero one-hot columns).
  - dma_gather indices are int16 (max 32767), so nodes are split into two
    sets A/B by their position within the owner's shard (local offset < 3200);
    gather sources are the correspondingly reordered xA/xB (host-permuted)
    and h1fullA/h1fullB.  The A half of the h1 AllGather is issued as soon as
    the first 25 blocks are done, overlapping the rest of layer 1.
"""

import numpy as np
import ml_dtypes

from concourse import bacc, bass, mybir, bass_utils
from concourse.bass import InstructionNameOrderedSet
from concourse.masks import make_identity
import concourse.tile as tile

N = 50000
E = 800000
F = 64          # feature width of x / h1 / h2
G = 128         # number of graphs
OUT = 8
P = 128
C = 8
NSH = N // C    # 6250 nodes per core
ABL = 3200      # A/B split point (local offset, 25 blocks)
NA = C * ABL            # rows in the A gather source (25600)
NBB = C * (NSH - ABL)   # rows in the B gather source (24400)
NB = (NSH + P - 1) // P   # 49 dst blocks per core
ABLK = ABL // P           # 25 blocks in A
SBLK = 4                  # dst blocks per supergather
NSB = (NB + SBLK - 1) // SBLK
GW = 2 * F                # gather row width in bf16 elements (256B rows)
F32 = mybir.dt.float32
BF16 = mybir.dt.bfloat16
I16 = mybir.dt.int16


def _bcast_ap(ap, dims):
    """Build a broadcast view of `ap` with explicit [step, count] dims."""
    return bass.AP(tensor=ap.tensor, offset=ap.offset, ap=dims)


def _ab_index(n):
    """Map global node id -> (set, idx-within-set) for the A/B split."""
    r, l = n // NSH, n % NSH
    s = l >= ABL
    return s, np.where(s, r * (NSH - ABL) + (l - ABL), r * ABL + l)


def _preprocess(src, dst, batch):
    """Host-side index preprocessing (pure integer/index work)."""
    src = np.asarray(src).astype(np.int64)
    dst = np.asarray(dst).astype(np.int64)
    batch = np.asarray(batch).astype(np.int64)

    deg = np.bincount(dst, minlength=N).astype(np.float64) + 1.0
    dinv = (1.0 / np.sqrt(deg)).astype(np.float32)
    st_all, sidx_all = _ab_index(src)
    st_all = st_all.astype(np.int64)

    # Per-core edge lists grouped by (block, set), and per-(core,block,set)
    # counts for the shared static plan.
    core_groups = []
    counts = np.zeros((C, NB, 2), np.int64)
    for c in range(C):
        lo = c * NSH
        m = (dst >= lo) & (dst < lo + NSH)
        es, ed = sidx_all[m], dst[m]
        st = st_all[m]
        dloc = ed - lo
        blk = dloc >> 7
        key = blk * 2 + st
        order = np.argsort(key, kind="stable")
        es, dloc, key = es[order], dloc[order], key[order]
        np.add.at(counts[c], (blk[order], st[order]), 1)
        core_groups.append((es, dloc, key))

    nch_bs = np.ceil(counts.max(axis=0) / P).astype(np.int64)  # [NB, 2]
    nch_bs = np.maximum(nch_bs, 1)

    # Static chunk-slot layout: per (sbi, s) gather, slots assigned
    # round-robin over blocks by chunk round r so per-core pure-padding
    # chunks cluster at the tail of every gather's index list.
    nch_sb = np.zeros((NSB, 2), np.int64)
    pos_of = {}          # (b, s, r) -> slot within its (sbi, s) gather
    slot_owner = {}      # (sbi, s) -> list of (b, r) per slot
    for sbi in range(NSB):
        blocks = list(range(sbi * SBLK, min((sbi + 1) * SBLK, NB)))
        for s in range(2):
            owners = []
            rmax = max(int(nch_bs[b, s]) for b in blocks)
            p = 0
            for r in range(rmax):
                for b in blocks:
                    if r < nch_bs[b, s]:
                        pos_of[(b, s, r)] = p
                        owners.append((b, r))
                        p += 1
            nch_sb[sbi, s] = p
            slot_owner[(sbi, s)] = owners

    chunk_base = {}
    idxcol_base = {}
    tot_chunks = 0
    idx_cols = [0, 0]
    for sbi in range(NSB):
        for s in range(2):
            chunk_base[(sbi, s)] = tot_chunks
            tot_chunks += int(nch_sb[sbi, s])
            idxcol_base[(sbi, s)] = idx_cols[s]
            idx_cols[s] += int(nch_sb[sbi, s]) * (P // 16)

    plan = dict(nch_bs=nch_bs, nch_sb=nch_sb, chunk_base=chunk_base,
                idxcol_base=idxcol_base, pos_of=pos_of,
                tot_chunks=tot_chunks, idx_cols=idx_cols)

    per_core = []
    for c in range(C):
        es, dloc, key = core_groups[c]
        bounds = np.searchsorted(key, np.arange(NB * 2 + 1))
        idx_parts = [[], []]
        dl_parts = []
        for sbi in range(NSB):
            for s in range(2):
                nch = int(nch_sb[sbi, s])
                gi = np.zeros(nch * P, np.int64)
                gd = np.full(nch * P, -1.0, np.float32)
                for b in range(sbi * SBLK, min((sbi + 1) * SBLK, NB)):
                    k = b * 2 + s
                    g0, g1 = bounds[k], bounds[k + 1]
                    for r in range(int(nch_bs[b, s])):
                        e0 = g0 + r * P
                        e1 = min(g0 + (r + 1) * P, g1)
                        if e1 <= e0:
                            continue
                        slot = pos_of[(b, s, r)]
                        cnt = e1 - e0
                        gi[slot * P:slot * P + cnt] = es[e0:e1]
                        gd[slot * P:slot * P + cnt] = dloc[e0:e1] - (b << 7)
                idx_parts[s].append(gi)
                dl_parts.append(gd)
        dstloc = np.concatenate(dl_parts).reshape(-1, P).T
        idx = []
        for s in range(2):
            stk = np.concatenate(idx_parts[s]).astype(np.int16)
            idx.append(np.tile(stk.reshape(-1, 16).T, (8, 1)))
        batchloc = np.full((P, NB), -1.0, np.float32)
        full = np.full(NB * P, -1.0, np.float32)
        full[:NSH] = batch[c * NSH:(c + 1) * NSH]
        batchloc[:, :] = full.reshape(NB, P).T
        dcol = np.zeros(NB * P, np.float32)
        dcol[:NSH] = dinv[c * NSH:(c + 1) * NSH]
        dcol = dcol.reshape(NB, P).T.copy()
        per_core.append(dict(
            idx0=idx[0], idx1=idx[1],
            dstloc=dstloc.astype(ml_dtypes.bfloat16),
            dinvcol=dcol, batchloc=batchloc.astype(ml_dtypes.bfloat16)))

    cnt = np.bincount(batch, minlength=G).astype(np.float32)
    invc = (1.0 / np.maximum(cnt, 1.0)).astype(np.float32)
    return plan, per_core, dinv, invc


def _build(plan):
    """Build the SPMD Bass program (identical for all cores)."""
    nch_bs = plan["nch_bs"]
    nch_sb = plan["nch_sb"]
    chunk_base = plan["chunk_base"]
    idxcol_base = plan["idxcol_base"]
    pos_of = plan["pos_of"]
    NCH = plan["tot_chunks"]
    icols = plan["idx_cols"]

    nc = bacc.Bacc("TRN2", target_bir_lowering=False, debug=False,
                   num_devices=C, num_swdge_queues=4)

    xA = nc.dram_tensor("xA", [NA, GW], BF16, kind="ExternalInput")
    xB = nc.dram_tensor("xB", [NBB, GW], BF16, kind="ExternalInput")
    xown2 = nc.dram_tensor("xown2", [NSH, F], F32, kind="ExternalInput")
    idx0 = nc.dram_tensor("idx0", [P, icols[0]], I16, kind="ExternalInput")
    idx1 = nc.dram_tensor("idx1", [P, icols[1]], I16, kind="ExternalInput")
    dstloc = nc.dram_tensor("dstloc", [P, NCH], BF16, kind="ExternalInput")
    dinvcol = nc.dram_tensor("dinvcol", [P, NB], F32, kind="ExternalInput")
    batchloc = nc.dram_tensor("batchloc", [P, NB], BF16, kind="ExternalInput")
    iota_in = nc.dram_tensor("iota", [P, P], BF16, kind="ExternalInput")
    W1a = nc.dram_tensor("W1a", [F + 1, F], BF16, kind="ExternalInput")
    W2a = nc.dram_tensor("W2a", [F + 1, F], BF16, kind="ExternalInput")
    Wfc = nc.dram_tensor("Wfc", [F, OUT], F32, kind="ExternalInput")
    bfcb = nc.dram_tensor("bfcb", [P, OUT], F32, kind="ExternalInput")
    invc_in = nc.dram_tensor("invc", [F, G], F32, kind="ExternalInput")
    out = nc.dram_tensor("out", [G, OUT], F32, kind="ExternalOutput")

    with tile.TileContext(nc) as tc:
        with (
            tc.tile_pool(name="const", bufs=1) as cp,
            tc.tile_pool(name="gpool", bufs=3) as gp,
            tc.tile_pool(name="spool", bufs=3) as sp,
            tc.tile_pool(name="epool", bufs=3) as ep,
            tc.tile_pool(name="psA", bufs=2, space="PSUM") as psA,
            tc.tile_pool(name="psB", bufs=1, space="PSUM") as psB,
            tc.tile_pool(name="dram", bufs=1, space="DRAM") as dram,
        ):
            # ---- constants / metadata loads ----
            iota_sb = cp.tile([P, P], BF16, tag="iota")
            nc.sync.dma_start(iota_sb[:], iota_in[:])
            ident = cp.tile([P, P], F32, tag="ident")
            make_identity(nc, ident[:])
            idx_sb = [cp.tile([P, icols[0]], I16, tag="idx0", name="idx_sb0"),
                      cp.tile([P, icols[1]], I16, tag="idx1", name="idx_sb1")]
            nc.scalar.dma_start(idx_sb[0][:], idx0[:])
            nc.scalar.dma_start(idx_sb[1][:], idx1[:])
            dl_sb = cp.tile([P, NCH], BF16, tag="dstloc")
            nc.scalar.dma_start(dl_sb[:], dstloc[:])
            dc_sb = cp.tile([P, NB], F32, tag="dinvcol")
            nc.sync.dma_start(dc_sb[:], dinvcol[:])
            bl_sb = cp.tile([P, NB], BF16, tag="batchloc")
            nc.scalar.dma_start(bl_sb[:], batchloc[:])
            W1_sb = cp.tile([F + 1, F], BF16, tag="W1a")
            nc.sync.dma_start(W1_sb[:], W1a[:])
            W2_sb = cp.tile([F + 1, F], BF16, tag="W2a")
            nc.sync.dma_start(W2_sb[:], W2a[:])
            Wfc_sb = cp.tile([F, OUT], F32, tag="Wfc")
            nc.sync.dma_start(Wfc_sb[:], Wfc[:])
            bfc_sb = cp.tile([P, OUT], F32, tag="bfcb")
            nc.sync.dma_start(bfc_sb[:], bfcb[:])
            invc_sb = cp.tile([F, G], F32, tag="invc")
            nc.sync.dma_start(invc_sb[:], invc_in[:])

            # layer-1 own rows (dinv^2 * x, f32) and layer-2 own rows
            # (dinv^2 * h1, written by the layer-1 epilogue; SBUF-resident)
            x_own = cp.tile([P, NB, F], F32, tag="x_own")
            nc.vector.memset(x_own[:, NB - 1, :], 0.0)
            nfull = NSH // P
            nc.sync.dma_start(
                x_own[:, 0:nfull, :],
                xown2[:nfull * P, :].rearrange("(b p) f -> p b f", p=P),
            )
            rem = NSH - nfull * P
            nc.sync.dma_start(x_own[:rem, nfull, :], xown2[nfull * P:NSH, :])
            h1own = cp.tile([P, NB, F], F32, tag="h1own")

            h1shardA = dram.tile([ABL, GW], BF16)
            h1shardB = dram.tile([NSH - ABL, GW], BF16)
            h1fullA = dram.tile([NA, GW], BF16, addr_space="Shared")
            h1fullB = dram.tile([NBB, GW], BF16, addr_space="Shared")
            pool_in = dram.tile([F, G], F32)
            pool_out = dram.tile([F, G], F32, addr_space="Shared")

            pool_ps = psB.tile([F, G], F32, tag="pool")

            # batched pool one-hots for all 49 blocks (generated at startup)
            Sp_all = cp.tile([P, NB, G], BF16, tag="Sp_all")
            blm = bl_sb[:, :]
            nc.vector.tensor_tensor(
                out=Sp_all[:],
                in0=_bcast_ap(iota_sb[:], [iota_sb[:].ap[0], [0, NB], [1, G]]),
                in1=_bcast_ap(blm, [blm.ap[0], [blm.ap[1][0], NB], [0, G]]),
                op=mybir.AluOpType.is_equal,
            )

            # Gathers rotate over the 4 SWDGE queues (desc-gen runs on a
            # different Q7 core pair per queue and can overlap).  Tile's
            # DMA-sem lanes are assigned round-robin over FINAL program
            # order and each sem is locked to one queue, so the rotation is
            # only valid if the scheduler cannot reorder gathers: chain them
            # with no-sync deps to pin program order = issue order.
            gq = [0]
            prev_gather = [None]
            dma_sems = [nc.alloc_semaphore(f"gdma{q}") for q in range(4)]

            def gather(t, src_ap, idx_tile, icol0, nidx, prep=False):
                q = gq[0] % 4
                gq[0] += 1
                inst = nc.gpsimd.dma_gather(
                    t[:], src_ap, idx_tile[:, icol0:icol0 + nidx // 16],
                    nidx, nidx, GW,
                    single_packet=False, queue_num=q,
                    prepare_only=prep, sem=dma_sems[q] if prep else None,
                )
                if prev_gather[0] is not None:
                    deps = InstructionNameOrderedSet()
                    deps.add(prev_gather[0])
                    inst.ins.add_nosync_dependencies_from(deps)
                prev_gather[0] = inst.ins.name
                return q

            def conv_layer(srcsAB, own_tile, W_sb, sink, prep=False):
                # Look-ahead per set: gathers are issued PRE{0,1} supergathers
                # before their consumption.  In prep mode (layer 2), gathers
                # are PREPARE_ONLY + trigger pairs: descriptor generation runs
                # on the Q7 during the AllGather wait (prep reads only idx
                # metadata); the source-data dependency defers to the trigger.
                PRE0 = 2
                PRE1 = 2 if prep else 0
                gtt = {0: {}, 1: {}}
                Stt = {0: {}, 1: {}}
                pend = []

                def issue(sbi, s):
                    nch = int(nch_sb[sbi, s])
                    if nch == 0:
                        gtt[s][sbi], Stt[s][sbi] = None, None
                        return
                    g = gp.tile([P, nch, GW], BF16, tag=f"g{s}",
                                bufs=(4 if s == 0 else 2))
                    q = gather(g, srcsAB[s], idx_sb[s],
                               idxcol_base[(sbi, s)], nch * P, prep=prep)
                    if prep:
                        pend.append(q)
                    cb = chunk_base[(sbi, s)]
                    S_t = sp.tile([P, nch, P], BF16, tag=f"S{s}",
                                  bufs=(4 if s == 0 else 2))
                    dmap = dl_sb[:, cb:cb + nch]
                    nc.vector.tensor_tensor(
                        out=S_t[:],
                        in0=_bcast_ap(iota_sb[:], [iota_sb[:].ap[0], [0, nch], [1, P]]),
                        in1=_bcast_ap(dmap, [dmap.ap[0], [dmap.ap[1][0], nch], [0, P]]),
                        op=mybir.AluOpType.is_equal,
                    )
                    gtt[s][sbi], Stt[s][sbi] = g, S_t

                def fire():
                    for q in pend:
                        nc.gpsimd.trigger_dma(count=None, queue_num=q)
                    pend.clear()

                for pre in range(min(PRE0, NSB)):
                    issue(pre, 0)
                for pre in range(min(PRE1, NSB)):
                    issue(pre, 1)
                fire()

                for sbi in range(NSB):
                    if sbi + PRE0 < NSB:
                        issue(sbi + PRE0, 0)
                        fire()
                    if sbi + PRE1 < NSB:
                        issue(sbi + PRE1, 1)
                        fire()
                    gt = {0: gtt[0].pop(sbi), 1: gtt[1].pop(sbi)}
                    St = {0: Stt[0].pop(sbi), 1: Stt[1].pop(sbi)}
                    for b in range(sbi * SBLK, min((sbi + 1) * SBLK, NB)):
                        agg_ps = psA.tile([P, F], F32, tag="agg")
                        tot = int(nch_bs[b, 0] + nch_bs[b, 1])
                        done = 0
                        for s in range(2):
                            for r in range(int(nch_bs[b, s])):
                                pos = pos_of[(b, s, r)]
                                nc.tensor.matmul(
                                    agg_ps[:], lhsT=St[s][:, pos, :],
                                    rhs=gt[s][:, pos, 0:F],
                                    start=(done == 0), stop=(done == tot - 1),
                                )
                                done += 1
                        # epilogue: h = tanh(((dinv*agg + own) @ Waug))
                        # (Waug carries the bias on an augmented ones-row)
                        aggS = ep.tile([P, F], F32, tag="aggS", bufs=6)
                        nc.scalar.activation(
                            aggS[:], agg_ps[:],
                            mybir.ActivationFunctionType.Copy,
                            scale=dc_sb[:, b:b + 1])
                        s1 = ep.tile([P, F], F32, tag="s1", bufs=6)
                        nc.vector.tensor_add(s1[:], aggS[:], own_tile[:, b, :])
                        trp = psA.tile([F, P], F32, tag="tr")
                        nc.tensor.transpose(trp[:], s1[:], ident[:])
                        aggT = ep.tile([F + 1, P], BF16, tag="aggT", bufs=6)
                        nc.vector.memset(aggT[F:F + 1, :], 1.0)
                        nc.scalar.activation(
                            aggT[0:F, :], trp[:],
                            mybir.ActivationFunctionType.Copy)
                        h_ps = psA.tile([P, F], F32, tag="h")
                        nc.tensor.matmul(h_ps[:], lhsT=aggT[:], rhs=W_sb[:],
                                         start=True, stop=True)
                        sink(b, h_ps)

            def sink1(b, h_ps):
                h1t = ep.tile([P, F], F32, tag="h1t", bufs=6)
                nc.scalar.activation(h1t[:], h_ps[:],
                                     mybir.ActivationFunctionType.Tanh)
                h1pr = ep.tile([P, F], BF16, tag="h1pr", bufs=6)
                nc.scalar.activation(h1pr[:], h1t[:],
                                     mybir.ActivationFunctionType.Copy,
                                     scale=dc_sb[:, b:b + 1])
                nc.scalar.activation(h1own[:, b, :], h1pr[:],
                                     mybir.ActivationFunctionType.Copy,
                                     scale=dc_sb[:, b:b + 1])
                if b < ABLK:
                    r0 = b * P
                    nc.sync.dma_start(h1shardA[r0:r0 + P, 0:F], h1pr[:])
                else:
                    r0 = (b - ABLK) * P
                    rows = min(P, (NSH - ABL) - r0)
                    nc.sync.dma_start(h1shardB[r0:r0 + rows, 0:F],
                                      h1pr[:rows, :])

            def sink2(b, h_ps):
                h2 = ep.tile([P, F], BF16, tag="h2", bufs=6)
                nc.scalar.activation(h2[:], h_ps[:],
                                     mybir.ActivationFunctionType.Tanh)
                nc.tensor.matmul(pool_ps[:], lhsT=h2[:], rhs=Sp_all[:, b, :],
                                 start=(b == 0), stop=(b == NB - 1),
                                 skip_group_check=True)

            conv_layer((xA[:], xB[:]), x_own, W1_sb, sink1)
            nc.gpsimd.collective_compute(
                "AllGather", mybir.AluOpType.bypass,
                ins=[h1shardA.opt()], outs=[h1fullA.opt()],
                replica_groups=[list(range(C))],
            )
            nc.gpsimd.collective_compute(
                "AllGather", mybir.AluOpType.bypass,
                ins=[h1shardB.opt()], outs=[h1fullB.opt()],
                replica_groups=[list(range(C))],
            )
            conv_layer((h1fullA[:], h1fullB[:]), h1own, W2_sb, sink2)

            # ---- pooled tail ----
            poolT = ep.tile([F, G], F32, tag="poolT")
            nc.vector.tensor_copy(poolT[:], pool_ps[:])
            nc.sync.dma_start(pool_in[:], poolT[:])
            nc.gpsimd.collective_compute(
                "AllReduce", mybir.AluOpType.add,
                ins=[pool_in.opt()], outs=[pool_out.opt()],
                replica_groups=[list(range(C))],
            )
            poolR = ep.tile([F, G], F32, tag="poolR")
            nc.sync.dma_start(poolR[:], pool_out[:])
            nc.vector.tensor_mul(poolR[:], poolR[:], invc_sb[:])
            fc_ps = psB.tile([G, OUT], F32, tag="fc")
            nc.tensor.matmul(fc_ps[:], lhsT=poolR[:], rhs=Wfc_sb[:],
                             start=True, stop=True)
            out_sb = ep.tile([G, OUT], F32, tag="out_sb")
            nc.vector.tensor_add(out_sb[:], fc_ps[:], bfc_sb[:])
            nc.sync.dma_start(out[:], out_sb[:])

    nc.compile()
    return nc


def _in_maps(plan, per_core, dinv, invc, x, W1, b1, W2, b2, Wfc, bfc):
    iota = np.tile(np.arange(P, dtype=np.float32), (P, 1)).astype(ml_dtypes.bfloat16)
    xf = np.asarray(x, np.float32)
    xs = np.zeros((N, GW), ml_dtypes.bfloat16)
    xs[:, :F] = (xf * dinv[:, None]).astype(ml_dtypes.bfloat16)
    xr = xs.reshape(C, NSH, GW)
    xA = np.ascontiguousarray(xr[:, :ABL, :].reshape(NA, GW))
    xB = np.ascontiguousarray(xr[:, ABL:, :].reshape(NBB, GW))
    xo2 = xf * (dinv[:, None] ** 2)

    def aug(W, b):
        Wa = np.zeros((F + 1, F), np.float32)
        Wa[:F] = np.asarray(W, np.float32)
        Wa[F] = np.asarray(b, np.float32)
        return Wa.astype(ml_dtypes.bfloat16)

    shared = dict(
        xA=xA, xB=xB,
        iota=iota,
        W1a=aug(W1, b1),
        W2a=aug(W2, b2),
        Wfc=np.ascontiguousarray(np.asarray(Wfc, np.float32)),
        bfcb=np.tile(np.asarray(bfc, np.float32), (P, 1)),
        invc=np.tile(invc, (F, 1)),
    )
    maps = []
    for c in range(C):
        m = dict(shared)
        m.update(per_core[c])
        m["xown2"] = xo2[c * NSH:(c + 1) * NSH]
        maps.append({k: np.ascontiguousarray(v) for k, v in m.items()})
    return maps


_RUN_KWARGS = {}


def kernel(x, src, dst, batch, W1, b1, W2, b2, Wfc, bfc):
    plan, per_core, dinv, invc = _preprocess(src, dst, batch)
    nc = _build(plan)
    maps = _in_maps(plan, per_core, dinv, invc, x, W1, b1, W2, b2, Wfc, bfc)
    res = bass_utils.run_bass_kernel_spmd(
        nc, maps, core_ids=list(range(C)), **_RUN_KWARGS
    )
    kernel.last_results = res
    return np.asarray(res.results[0]["out"], np.float32)
